# revision 7
# baseline (speedup 1.0000x reference)
"""Trainium2 Bass kernel for Bengio03HighwayBiLm.

Model: L=2 layers x 2 directions of [width-4 conv over sequence (H=512 -> 512)
+ ReLU + 2 highway sublayers (512 -> 1024 split into nonlin/gate)].

Sharding: data-parallel over batch across 8 cores (4 batches/core), weights
replicated. On device everything runs in channels-on-partitions layout
([ch, pos]); the host pre-transposes the input / weights and post-transposes
the output, so the device kernel needs no transposes at all. The conv is 4
accumulated matmuls over a column-shifted padded activation window.

Precision: layer-0 conv + highway-nonlin run in float16 (full PE rate).
All of layer 1 AND layer-0's highway gate halves run in float8e4 with
perf_mode=DoubleRow (2 fp8 weights per PE cell, K=256 per instruction ->
2x rate; measured same 216ns as a K=128 fp16 matmul at FD=512). The gate
halves tolerate fp8 because sigmoid attenuates gate-path quantization
noise. fp8 operands carry power-of-2 scales (weights 2^9 / 2^8,
activations 2^3) undone for free via activation `scale` operands; PSUM
accumulates fp32. End-to-end rel-rms error ~1.67e-2 vs the 2e-2 budget;
adding any more fp8 overruns the budget. The layer-1 combine runs in the
D = HWS*XS domain so VectorE can drain the nonlin PSUMs with a plain
2-op tensor_scalar; the host divides the layer-1 output by D.

Schedule: iterations are split into chunks (conv / highway-0 / highway-1)
and chunks of independent iterations are software-pipelined so the
combine->fp8-cast latency at each highway boundary is hidden under another
iteration's matmuls. Inter-layer fp8 activations never touch DRAM: the
layer-0 combine's fp8 cast writes straight into padded DoubleRow pair
tiles in SBUF.

Highway combines run on [128, 2, 512] half-tiles of per-chunk [128, 4, 512]
activation tiles (conv/r/g/x all share the big-tile layout), so each
combine is 3 FD=1024 tensor_tensor ops instead of 6 FD=512 ones (the DVE
fixed ~120-cycle instruction overhead halves) and each output is a single
DMA. The very last highway chunk routes its second-half combine to the
otherwise-idle GpSimd engine to shorten the post-matmul vector drain.

Startup: the framework preamble owns the first ~6us; tiny self-matmuls on
a memset tile then keep the PE HAM clock gate warm (cold = 1.2GHz) while
the first DMAs land. Weights live in a handful of large consolidated DRAM
tensors (~20 big DMAs instead of ~250 small ones; descriptor issue costs
~650ns each on a queue). Startup-critical tiles (input b=0, the four
layer-0 d0 conv tap tiles, layer-0 highway weights) go on the two HWDGE
queues (sync + scalar) which move data promptly; everything needed after
~90us rides the slower gpsimd SWDGE queue.
"""

import sys

for _p in ("/opt/trn_rl_repo", "/root/.axon_site/_ro/trn_rl_repo"):
    if _p not in sys.path:
        sys.path.append(_p)

from contextlib import ExitStack

import numpy as np
import ml_dtypes

import concourse.bass as bass
import concourse.tile as tile
from concourse import bacc, bass_utils, mybir

F32 = mybir.dt.float32
F16 = mybir.dt.float16
F8 = mybir.dt.float8e4
AF = mybir.ActivationFunctionType
ALU = mybir.AluOpType
DR = mybir.MatmulPerfMode.DoubleRow
E4NP = ml_dtypes.float8_e4m3

B, S, H = 32, 512, 512
L, NHW, WIDTH = 2, 2, 3
NCORES = 8
BL = B // NCORES          # batches per core
SP = S + 2 * WIDTH        # padded sequence length (fp16 layer-0 input)
SP8 = 528                 # fp8 padded length (DoubleRow needs stride % 16 == 0)
HC = H // 128             # channel chunks

# fp8 operand scales (powers of 2; undone in activation `scale`)
XS = 2.0 ** 3             # activations
CWS = 2.0 ** 9            # layer-1 conv weights
HWS = 2.0 ** 8            # layer-1 highway weights

# bias_all column map: [cb | cb8l0 | cb11 | cb8 | hb | hb1 | hb11]
B_CB, B_CB8L0, B_CB11, B_CB8 = 0, 4, 8, 12
B_HB, B_HB1, B_HB11 = 16, 32, 48

_CACHE = {}


def _build():
    if "nc" in _CACHE:
        return _CACHE["nc"]

    nc = bacc.Bacc("TRN2", target_bir_lowering=False, debug=False,
                   num_devices=NCORES)

    x_t = nc.dram_tensor("x_t", [BL, 128, HC, SP], F16,
                         kind="ExternalInput").ap()
    convw = nc.dram_tensor("convw", [2, 4, 128, HC, 512], F16,
                           kind="ExternalInput").ap()          # layer 0
    hww = nc.dram_tensor("hww", [2, 128, NHW, 4, HC, 128], F16,
                         kind="ExternalInput").ap()            # layer 0 nonlin
    convw8 = nc.dram_tensor("convw8", [2, 128, 4, 2, 2, 512], F8,
                            kind="ExternalInput").ap()         # layer 1
    hww8 = nc.dram_tensor("hww8", [2, 128, NHW, 8, 2, 2, 128], F8,
                          kind="ExternalInput").ap()           # layer 1
    hwg8 = nc.dram_tensor("hwg8", [2, 128, NHW, 4, 2, 2, 128], F8,
                          kind="ExternalInput").ap()           # L0 gate half
    bias_all = nc.dram_tensor("bias_all", [128, 2, 64], F32,
                              kind="ExternalInput").ap()
    padt8r = nc.dram_tensor("padt8r", [128, 2, BL, 2, 2, WIDTH], F8,
                            kind="ExternalInput").ap()         # layer-1 pads
    out_t = nc.dram_tensor("out_t", [L, 2, BL, 128, HC, S], F16,
                           kind="ExternalOutput").ap()

    with tile.TileContext(nc) as tc, ExitStack() as ctx:
        sb = ctx.enter_context(tc.tile_pool(name="sb", bufs=2))
        ps = ctx.enter_context(tc.tile_pool(name="ps", bufs=8, space="PSUM"))

        # ---- PE warmup: tiny self-matmuls on a memset tile keep the HAM
        # activity window busy while the startup DMAs fly, so the first
        # real matmuls run at 2.4GHz instead of the cold 1.2GHz.
        warm = sb.tile([128, 128], F16, name="warm", tag="warm", bufs=1)
        nc.vector.memset(warm[:], 0.0)
        wps = ps.tile([128, 512], F32, name="warmps", tag="ps")
        for _ in range(44):
            nc.tensor.matmul(wps[:, 0:128], warm[:], warm[:],
                             start=True, stop=True)

        # ---- persistent SBUF state ------------------------------------
        # inter-layer fp8 activations (XS-scaled), padded DoubleRow pair
        # tiles living entirely in SBUF: [ki, d, b, a, ko, pos]
        xsb = sb.tile([128, 2, BL, 2, 2, SP8], F8, name="xsb", tag="xsb",
                      bufs=1)

        xin_t = [None] * BL

        def load_xin(b, eng):
            t = sb.tile([128, HC, SP], F16, name=f"xin_{b}", tag="xin",
                        bufs=BL)
            eng.dma_start(t[:], x_t[b])
            xin_t[b] = t

        wc = [[None] * 4 for _ in range(2)]       # [d][j] -> [128, HC, 512]

        def load_wc(d, j, eng):
            w = sb.tile([128, HC, 512], F16, name=f"wc_{d}{j}", tag="wc",
                        bufs=8)
            eng.dma_start(w[:], convw[d, j])
            wc[d][j] = w

        wh = [None, None]                         # [128, NHW, 4, HC, 128]
        whg8 = [None, None]                       # [128, NHW, 4, 2, 2, 128]
        wc8 = [None, None]                        # [128, 4, 2, 2, 512]
        wh8 = [None, None]                        # [128, NHW, 8, 2, 2, 128]

        # ---- DMA issue schedule ---------------------------------------
        # Startup-critical tiles on the two prompt HWDGE queues (sync +
        # scalar), in first-use order; everything needed later than ~90us
        # on the gpsimd SWDGE queue (slower ring, doesn't matter there).
        load_xin(0, nc.sync)
        load_wc(0, 0, nc.scalar)
        load_wc(0, 2, nc.sync)
        load_wc(0, 1, nc.scalar)
        load_wc(0, 3, nc.scalar)

        btile = sb.tile([128, 2, 64], F32, name="btile", tag="btile", bufs=1)
        nc.gpsimd.dma_start(btile[:], bias_all[:])

        wh[0] = sb.tile([128, NHW, 4, HC, 128], F16, name="wh_0", tag="wh",
                        bufs=2)
        nc.sync.dma_start(wh[0][:], hww[0])
        whg8[0] = sb.tile([128, NHW, 4, 2, 2, 128], F8, name="whg8_0",
                          tag="whg8", bufs=2)
        nc.scalar.dma_start(whg8[0][:], hwg8[0])

        for b in range(1, BL):
            load_xin(b, nc.sync)

        # layer-1 pad columns: one tiny staged DMA, then vector copies
        # into the xsb pair tiles (a strided DMA straight into the pad
        # columns explodes into thousands of 3-byte descriptors).
        padstage = sb.tile([128, 2, BL, 2, 2, WIDTH], F8, name="padstage",
                           tag="padstage", bufs=1)
        nc.gpsimd.dma_start(padstage[:], padt8r[:])
        for d in range(2):
            for a in range(2):
                nc.vector.tensor_copy(xsb[:, d, :, a, :, 0:WIDTH],
                                      padstage[:, 0, :, a])
                nc.vector.tensor_copy(
                    xsb[:, d, :, a, :, WIDTH + S:WIDTH + S + WIDTH],
                    padstage[:, 1, :, a])

        for j in range(4):
            load_wc(1, j, nc.sync)
        wh[1] = sb.tile([128, NHW, 4, HC, 128], F16, name="wh_1", tag="wh",
                        bufs=2)
        nc.sync.dma_start(wh[1][:], hww[1])
        whg8[1] = sb.tile([128, NHW, 4, 2, 2, 128], F8, name="whg8_1",
                          tag="whg8", bufs=2)
        nc.gpsimd.dma_start(whg8[1][:], hwg8[1])
        for d in range(2):
            wc8[d] = sb.tile([128, 4, 2, 2, 512], F8, name=f"wc8_{d}",
                             tag="wc8", bufs=2)
            nc.gpsimd.dma_start(wc8[d][:], convw8[d])
            wh8[d] = sb.tile([128, NHW, 8, 2, 2, 128], F8, name=f"wh8_{d}",
                             tag="wh8", bufs=2)
            nc.gpsimd.dma_start(wh8[d][:], hww8[d])

        def bias(d, base, i):
            return btile[:, d, base + i:base + i + 1]

        # ---- layer-0 iteration chunks (fp16) --------------------------
        def l0_chunks(d, b):
            st = {}
            off = 0 if d == 0 else WIDTH

            def conv():
                xin = xin_t[b]
                xbig = sb.tile([128, HC, 512], F16, name=f"hf_{d}{b}",
                               tag="hf", bufs=3)
                h8p = [None, None]
                for oc in range(4):
                    pt = ps.tile([128, 512], F32, name=f"cps_{d}{b}{oc}",
                                 tag="ps")
                    k = 0
                    for j in range(4):
                        for hc in range(HC):
                            nc.tensor.matmul(
                                pt[:],
                                wc[d][j][:, hc, oc * 128:(oc + 1) * 128],
                                xin[:, hc, off + j:off + j + S],
                                start=(k == 0), stop=(k == 15))
                            k += 1
                    # fp16 drain FIRST: highway-0's nonlin matmuls read it
                    nc.scalar.activation(xbig[:, oc, :], pt[:], AF.Relu,
                                         bias=bias(d, B_CB, oc))
                    if h8p[oc // 2] is None:
                        h8p[oc // 2] = sb.tile(
                            [128, 2, 512], F8, name=f"h8l0_{d}{b}{oc // 2}",
                            tag="h8", bufs=8)
                    # XS-scaled fp8 copy feeds the fp8 gate matmuls
                    nc.scalar.activation(h8p[oc // 2][:, oc % 2, :], pt[:],
                                         AF.Relu, bias=bias(d, B_CB8L0, oc),
                                         scale=XS)
                st["x"] = xbig
                st["h8"] = h8p

            def hw(jh):
                xbig, h8p = st["x"], st["h8"]
                rbig = sb.tile([128, HC, 512], F16, tag="rt", bufs=2,
                               name=f"rt_{d}{b}{jh}")
                gbig = sb.tile([128, HC, 512], F16, tag="gt", bufs=2,
                               name=f"gt_{d}{b}{jh}")
                xobig = sb.tile([128, HC, 512], F16, tag=f"xo{jh}", bufs=3,
                                name=f"xo_{d}{b}{jh}")
                h8p_new = [None, None]
                for half in range(2):
                    for gc in (0, 4, 1, 5) if half == 0 else (2, 6, 3, 7):
                        pt = ps.tile([128, 512], F32,
                                     tag="ps", name=f"hps_{d}{b}{jh}{gc}")
                        bi = jh * 8 + gc
                        if gc < 4:
                            for hc in range(HC):
                                nc.tensor.matmul(
                                    pt[:], wh[d][:, jh, gc, hc, :],
                                    xbig[:, hc, :],
                                    start=(hc == 0), stop=(hc == HC - 1))
                            nc.scalar.activation(rbig[:, gc, :], pt[:],
                                                 AF.Relu,
                                                 bias=bias(d, B_HB, bi))
                        else:
                            # gate half in fp8 DoubleRow: PSUM = HWS*XS*z
                            for a in range(2):
                                nc.tensor.matmul(
                                    pt[:], whg8[d][:, jh, gc - 4, a, :, :],
                                    h8p[a][:],
                                    start=(a == 0), stop=(a == 1),
                                    perf_mode=DR)
                            nc.scalar.activation(gbig[:, gc - 4, :], pt[:],
                                                 AF.Sigmoid,
                                                 bias=bias(d, B_HB, bi),
                                                 scale=1.0 / (HWS * XS))
                    hs = slice(2 * half, 2 * half + 2)
                    nc.vector.tensor_sub(xobig[:, hs, :], xbig[:, hs, :],
                                         rbig[:, hs, :])
                    nc.vector.tensor_mul(xobig[:, hs, :], gbig[:, hs, :],
                                         xobig[:, hs, :])
                    nc.vector.tensor_add(xobig[:, hs, :], xobig[:, hs, :],
                                         rbig[:, hs, :])
                    if jh < NHW - 1:
                        h8p_new[half] = sb.tile(
                            [128, 2, 512], F8, tag="h8", bufs=8,
                            name=f"x18l0_{d}{b}{half}")
                        nc.vector.tensor_scalar_mul(
                            h8p_new[half][:], xobig[:, hs, :], XS)
                    else:
                        # XS-scaled fp8 copy straight into the padded
                        # layer-1 input pair tile (no DRAM roundtrip)
                        nc.vector.tensor_scalar_mul(
                            xsb[:, d, b, half, :, WIDTH:WIDTH + S],
                            xobig[:, hs, :], XS)
                st["x"] = xobig
                if jh < NHW - 1:
                    st["h8"] = h8p_new
                else:
                    nc.gpsimd.dma_start(out_t[0, d, b], xobig[:])

            return conv, (lambda: hw(0)), (lambda: hw(1))

        # ---- layer-1 iteration chunks (fp8 DoubleRow) -----------------
        def l1_chunks(d, b, tail=False):
            st = {}
            off = 0 if d == 0 else WIDTH

            def conv():
                xbig = sb.tile([128, HC, 512], F16, name=f"hf1_{d}{b}",
                               tag="hf", bufs=3)
                h8p = [None, None]        # fp8 XS-scaled pair tiles
                for oc in range(4):
                    pt = ps.tile([128, 512], F32, name=f"cps1_{d}{b}{oc}",
                                 tag="ps")
                    k = 0
                    for j in range(4):
                        for a in range(2):
                            nc.tensor.matmul(
                                pt[:],
                                wc8[d][:, j, a, :, oc * 128:(oc + 1) * 128],
                                xsb[:, d, b, a, :, off + j:off + j + S],
                                start=(k == 0), stop=(k == 7), perf_mode=DR)
                            k += 1
                    if h8p[oc // 2] is None:
                        h8p[oc // 2] = sb.tile(
                            [128, 2, 512], F8, name=f"h8_{d}{b}{oc // 2}",
                            tag="h8", bufs=8)
                    # fp8 copy FIRST -- it feeds the next matmuls.
                    # XS*h = relu(psum/CWS + XS*bias)  (PSUM = CWS*XS*z)
                    nc.scalar.activation(h8p[oc // 2][:, oc % 2, :], pt[:],
                                         AF.Relu, bias=bias(d, B_CB8, oc),
                                         scale=XS / (CWS * XS))
                    # D*h = relu(psum*(HWS/CWS) + D*bias)
                    nc.scalar.activation(xbig[:, oc, :], pt[:], AF.Relu,
                                         bias=bias(d, B_CB11, oc),
                                         scale=HWS / CWS)
                st["x"] = xbig
                st["h8"] = h8p

            def hw(jh):
                xbig, h8p = st["x"], st["h8"]
                rbig = sb.tile([128, HC, 512], F16, tag="rt", bufs=2,
                               name=f"rt1_{d}{b}{jh}")
                gbig = sb.tile([128, HC, 512], F16, tag="gt", bufs=2,
                               name=f"gt1_{d}{b}{jh}")
                xobig = sb.tile([128, HC, 512], F16, tag=f"xo{jh}", bufs=3,
                                name=f"xo1_{d}{b}{jh}")
                h8p_new = [None, None]
                final = jh == NHW - 1
                for half in range(2):
                    for gc in (0, 4, 1, 5) if half == 0 else (2, 6, 3, 7):
                        pt = ps.tile([128, 512], F32,
                                     tag="ps", name=f"hps1_{d}{b}{jh}{gc}")
                        for a in range(2):
                            nc.tensor.matmul(
                                pt[:], wh8[d][:, jh, gc, a, :, :],
                                h8p[a][:],
                                start=(a == 0), stop=(a == 1), perf_mode=DR)
                        bi = jh * 8 + gc
                        if gc < 4:
                            # VectorE drains the nonlin PSUMs: D*r
                            nc.vector.tensor_scalar(
                                rbig[:, gc, :], pt[:], bias(d, B_HB11, bi),
                                0.0, ALU.add, ALU.max)
                        else:
                            nc.scalar.activation(gbig[:, gc - 4, :], pt[:],
                                                 AF.Sigmoid,
                                                 bias=bias(d, B_HB1, bi),
                                                 scale=1.0 / (HWS * XS))
                    hs = slice(2 * half, 2 * half + 2)
                    # the last chunk's second half combines on the idle
                    # GpSimd engine so the post-matmul drain is parallel
                    veng = nc.gpsimd if (tail and final and half) else \
                        nc.vector
                    veng.tensor_sub(xobig[:, hs, :], xbig[:, hs, :],
                                    rbig[:, hs, :])
                    veng.tensor_mul(xobig[:, hs, :], gbig[:, hs, :],
                                    xobig[:, hs, :])
                    veng.tensor_add(xobig[:, hs, :], xobig[:, hs, :],
                                    rbig[:, hs, :])
                    if not final:
                        h8p_new[half] = sb.tile(
                            [128, 2, 512], F8, tag="h8", bufs=8,
                            name=f"x18_{d}{b}{half}")
                        # back to XS-domain fp8 on the ScalarE
                        nc.scalar.activation(h8p_new[half][:],
                                             xobig[:, hs, :],
                                             AF.Copy, scale=1.0 / HWS)
                    elif tail:
                        # split the final output DMA across two queues;
                        # half 0 goes on sync so the waiting dma_start
                        # doesn't block gpsimd's half-1 combine ops
                        eng = nc.sync if half == 0 else nc.gpsimd
                        eng.dma_start(out_t[1, d, b, :, hs, :],
                                      xobig[:, hs, :])
                st["x"] = xobig
                if not final:
                    st["h8"] = h8p_new
                elif not tail:
                    nc.gpsimd.dma_start(out_t[1, d, b], xobig[:])

            return conv, (lambda: hw(0)), (lambda: hw(1))

        # ---- software-pipelined schedule ------------------------------
        # phase A: layer-0 d=0 (tensor-heavy; scalar/vector have slack).
        # hw(0) of iteration b and hw(1) of b are separated by conv(b+1)
        # so the combine->cast latency at the sublayer boundary is hidden.
        pa = [l0_chunks(0, b) for b in range(BL)]
        pa[0][0]()                       # c0
        pa[0][1]()                       # j0_0
        for b in range(1, BL):
            pa[b][0]()                   # c(b)
            pa[b - 1][2]()               # j1_(b-1)
            pa[b][1]()                   # j0_b
        # prologue: L = l0d1(0), A = l1d0(0); L's chunks cover A's
        # combine latencies (and phase A's last hw1)
        Lc, Lj0, Lj1 = l0_chunks(1, 0)
        Ac, Aj0, Aj1 = l1_chunks(0, 0)
        Lc(); pa[BL - 1][2](); Ac(); Lj0(); Aj0(); Lj1(); Aj1()
        # steady state: per group, L = l0d1(b), A = l1d0(b),
        # B = l1d1(b-1); B's last highway spills into the next group
        pending = None
        pending_Aj1 = None
        for b in range(1, BL):
            last = b == BL - 1
            Lc, Lj0, Lj1 = l0_chunks(1, b)
            Ac, Aj0, Aj1 = l1_chunks(0, b)
            Bc, Bj0, Bj1 = l1_chunks(1, b - 1)
            Lc()
            if pending is not None:
                pending()
            Ac(); Lj0(); Aj0(); Bc()
            if not last:
                Aj1()
            Lj1(); Bj0()
            pending = Bj1
            if last:
                pending_Aj1 = Aj1
        # epilogue: E = l1d1(3). Deferred chunks (Aj1 of b=3, Bj1 of b=2)
        # are slotted between E's chunks so each combine->cast latency is
        # covered by another chunk's matmuls.
        Ec, Ej0, Ej1 = l1_chunks(1, BL - 1, tail=True)
        Ec()
        pending_Aj1()
        Ej0()
        if pending is not None:
            pending()
        Ej1()

    nc.compile()
    _CACHE["nc"] = nc
    return nc


def _prep_shared(fwd_pad, bwd_pad, fwd_w, fwd_b, bwd_w, bwd_b,
                 fwd_hw_w, fwd_hw_b, bwd_hw_w, bwd_hw_b):
    f32 = np.float32
    convw = np.empty((2, 4, 128, HC, 512), np.float16)
    convw8 = np.empty((2, 128, 4, 2, 2, 512), E4NP)
    hww = np.empty((2, 128, NHW, 4, HC, 128), np.float16)
    hww8 = np.empty((2, 128, NHW, 8, 2, 2, 128), E4NP)
    hwg8 = np.empty((2, 128, NHW, 4, 2, 2, 128), E4NP)
    bias_all = np.empty((128, 2, 64), f32)
    padt8r = np.empty((128, 2, BL, 2, 2, WIDTH), E4NP)
    for d, (w, bia, hw_w, hw_b) in enumerate(
            [(fwd_w, fwd_b, fwd_hw_w, fwd_hw_b),
             (bwd_w, bwd_b, bwd_hw_w, bwd_hw_b)]):
        # layer 0 fp16: w[0]: [512o, 2048=(j,hc,p)] -> [j, p, hc, o]
        convw[d] = w[0].reshape(512, 4, HC, 128).transpose(1, 3, 2, 0)
        # layer 1 fp8 DoubleRow: [512o, (j,a,ko,ki)] -> [ki, j, a, ko, o]
        convw8[d] = (w[1].reshape(512, 4, 2, 2, 128)
                     .transpose(4, 1, 2, 3, 0) * CWS).astype(E4NP)
        cb = [bia[li].reshape(4, 128).T for li in range(L)]
        bias_all[:, d, B_CB:B_CB + 4] = cb[0]
        bias_all[:, d, B_CB8L0:B_CB8L0 + 4] = cb[0] * XS
        bias_all[:, d, B_CB11:B_CB11 + 4] = cb[1] * (HWS * XS)
        bias_all[:, d, B_CB8:B_CB8 + 4] = cb[1] * XS
        for li, base in ((0, B_HB), (1, B_HB1)):
            for jh in range(NHW):
                bias_all[:, d, base + jh * 8:base + (jh + 1) * 8] = \
                    hw_b[li, jh].reshape(8, 128).T
        for jh in range(NHW):
            bias_all[:, d, B_HB11 + jh * 8:B_HB11 + (jh + 1) * 8] = \
                hw_b[1, jh].reshape(8, 128).T * (HWS * XS)
        for jh in range(NHW):
            # layer-0 nonlin half: [512=(gc,gi), 512=(hc,p)] -> [p,gc,hc,gi]
            hww[d, :, jh] = (hw_w[0, jh][:H].reshape(4, 128, HC, 128)
                             .transpose(3, 0, 2, 1))
            # layer 1 fp8: [(gc,m), (a,ko,ki)] -> [ki, gc, a, ko, m]
            hww8[d, :, jh] = (hw_w[1, jh].reshape(8, 128, 2, 2, 128)
                              .transpose(4, 0, 2, 3, 1) * HWS).astype(E4NP)
            # layer-0 gate half (rows H:2H) in the same fp8 layout
            hwg8[d, :, jh] = (hw_w[0, jh][H:].reshape(4, 128, 2, 2, 128)
                              .transpose(4, 0, 2, 3, 1) * HWS).astype(E4NP)
    # layer-1 pads: [ki, front/back, b(replicated), a, ko, 3], XS-scaled
    for i, pad in enumerate((fwd_pad, bwd_pad)):
        p = (np.asarray(pad)[1].T.reshape(2, 2, 128, WIDTH)
             .transpose(2, 0, 1, 3) * XS).astype(E4NP)      # [ki, a, ko, 3]
        padt8r[:, i] = np.broadcast_to(p[:, None], (128, BL, 2, 2, WIDTH))
    return dict(convw=convw, convw8=convw8, hww=hww, hww8=hww8, hwg8=hwg8,
                bias_all=bias_all, padt8r=padt8r)


def kernel(inputs, fwd_pad, bwd_pad, fwd_w, fwd_b, bwd_w, bwd_b,
           fwd_hw_w, fwd_hw_b, bwd_hw_w, bwd_hw_b, _trace=False):
    nc = _build()
    shared = _prep_shared(
        np.asarray(fwd_pad), np.asarray(bwd_pad),
        np.asarray(fwd_w), np.asarray(fwd_b),
        np.asarray(bwd_w), np.asarray(bwd_b),
        np.asarray(fwd_hw_w), np.asarray(fwd_hw_b),
        np.asarray(bwd_hw_w), np.asarray(bwd_hw_b))
    x = np.asarray(inputs, dtype=np.float32)

    in_maps = []
    for c in range(NCORES):
        xs = x[c * BL:(c + 1) * BL].transpose(0, 2, 1)  # [BL, H, S]
        xc = np.empty((BL, H, SP), np.float16)
        xc[:, :, WIDTH:WIDTH + S] = xs
        xc[:, :, 0:WIDTH] = np.asarray(fwd_pad)[0].T[None]
        xc[:, :, WIDTH + S:SP] = np.asarray(bwd_pad)[0].T[None]
        # [BL, H, SP] -> [BL, 128, HC, SP]
        xc = np.ascontiguousarray(
            xc.reshape(BL, HC, 128, SP).transpose(0, 2, 1, 3))
        in_maps.append({"x_t": xc, **shared})

    res = bass_utils.run_bass_kernel_spmd(
        nc, in_maps, core_ids=list(range(NCORES)), trace=_trace)

    out = np.empty((L, B, S, 2 * H), np.float32)
    for c in range(NCORES):
        o = res.results[c]["out_t"].astype(np.float32)
        o[1] /= HWS * XS   # layer 1 is computed in the HWS*XS domain
        # o: [L, 2, BL, 128, HC, S] -> [BL, S, H] per (L, dir)
        for li in range(L):
            out[li, c * BL:(c + 1) * BL, :, :H] = \
                o[li, 0].transpose(0, 3, 2, 1).reshape(BL, S, H)
            out[li, c * BL:(c + 1) * BL, :, H:] = \
                o[li, 1].transpose(0, 3, 2, 1).reshape(BL, S, H)
    if _trace:
        kernel.last_exec_time_ns = res.exec_time_ns
        kernel.last_trace = (res.instructions_and_trace[1]
                             if res.instructions_and_trace else None)
        kernel.last_res = res
    return out


# revision 14
# speedup vs baseline: 1.0558x; 1.0558x over previous
"""Trainium2 Bass kernel for Bengio03HighwayBiLm.

Model: L=2 layers x 2 directions of [width-4 conv over sequence (H=512 -> 512)
+ ReLU + 2 highway sublayers (512 -> 1024 split into nonlin/gate)].

Sharding: data-parallel over batch across 8 cores (4 batches/core), weights
replicated. On device everything runs in channels-on-partitions layout
([ch, pos]); the host pre-transposes the input / weights and post-transposes
the output, so the device kernel needs no transposes at all. The conv is 4
accumulated matmuls over a column-shifted padded activation window.

Precision: layer-0 conv + highway-nonlin run in float16 (full PE rate).
All of layer 1 AND layer-0's highway gate halves run in float8e4 with
perf_mode=DoubleRow (2 fp8 weights per PE cell, K=256 per instruction ->
2x rate; measured same 216ns as a K=128 fp16 matmul at FD=512). The gate
halves tolerate fp8 because sigmoid attenuates gate-path quantization
noise. fp8 operands carry power-of-2 scales (weights 2^9 / 2^8,
activations 2^3) undone for free via activation `scale` operands; PSUM
accumulates fp32. End-to-end rel-rms error ~1.67e-2 vs the 2e-2 budget;
adding any more fp8 overruns the budget. The layer-1 combine runs in the
D = HWS*XS domain so VectorE can drain the nonlin PSUMs with a plain
2-op tensor_scalar; the host divides the layer-1 output by D.

Schedule: iterations are split into chunks (conv / highway-0 / highway-1)
and chunks of independent iterations are software-pipelined so the
combine->fp8-cast latency at each highway boundary is hidden under another
iteration's matmuls. Inter-layer fp8 activations never touch DRAM: the
layer-0 combine's fp8 cast writes straight into padded DoubleRow pair
tiles in SBUF.

Highway combines run on [128, 2, 512] half-tiles of per-chunk [128, 4, 512]
activation tiles (conv/r/g/x all share the big-tile layout), so each
combine is 3 FD=1024 tensor_tensor ops instead of 6 FD=512 ones (the DVE
fixed ~120-cycle instruction overhead halves) and each output is a single
DMA. The very last highway chunk routes its second-half combine to the
otherwise-idle GpSimd engine to shorten the post-matmul vector drain.

Startup: the framework preamble owns the first ~6us; tiny self-matmuls on
a memset tile then keep the PE HAM clock gate warm (cold = 1.2GHz) while
the first DMAs land. Weights live in a handful of large consolidated DRAM
tensors (~20 big DMAs instead of ~250 small ones; descriptor issue costs
~650ns each on a queue). Startup-critical tiles (input b=0, the four
layer-0 d0 conv tap tiles, layer-0 highway weights) go on the two HWDGE
queues (sync + scalar) which move data promptly; everything needed after
~90us rides the slower gpsimd SWDGE queue.
"""

import sys

for _p in ("/opt/trn_rl_repo", "/root/.axon_site/_ro/trn_rl_repo"):
    if _p not in sys.path:
        sys.path.append(_p)

from contextlib import ExitStack

import numpy as np
import ml_dtypes

import concourse.bass as bass
import concourse.tile as tile
from concourse import bacc, bass_utils, mybir

F32 = mybir.dt.float32
F16 = mybir.dt.float16
F8 = mybir.dt.float8e4
AF = mybir.ActivationFunctionType
ALU = mybir.AluOpType
DR = mybir.MatmulPerfMode.DoubleRow
E4NP = ml_dtypes.float8_e4m3

B, S, H = 32, 512, 512
L, NHW, WIDTH = 2, 2, 3
NCORES = 8
BL = B // NCORES          # batches per core
SP = S + 2 * WIDTH        # padded sequence length (fp16 layer-0 input)
SP8 = 528                 # fp8 padded length (DoubleRow needs stride % 16 == 0)
HC = H // 128             # channel chunks

# fp8 operand scales (powers of 2; undone in activation `scale`)
XS = 2.0 ** 3             # activations
CWS = 2.0 ** 9            # layer-1 conv weights
HWS = 2.0 ** 8            # layer-1 highway weights

# bias_all column map: [cb | cb8l0 | cb11 | cb8 | hb | hb1 | hb11]
B_CB, B_CB8L0, B_CB11, B_CB8 = 0, 4, 8, 12
B_HB, B_HB1, B_HB11 = 16, 32, 48

_CACHE = {}


def _build():
    if "nc" in _CACHE:
        return _CACHE["nc"]

    nc = bacc.Bacc("TRN2", target_bir_lowering=False, debug=False,
                   num_devices=NCORES)

    x_t = nc.dram_tensor("x_t", [BL, 128, HC, SP], F16,
                         kind="ExternalInput").ap()
    convw = nc.dram_tensor("convw", [2, 4, 128, HC, 512], F16,
                           kind="ExternalInput").ap()          # layer 0
    hww = nc.dram_tensor("hww", [2, 128, NHW, 4, HC, 128], F16,
                         kind="ExternalInput").ap()            # layer 0 nonlin
    convw8 = nc.dram_tensor("convw8", [2, 128, 4, 2, 2, 512], F8,
                            kind="ExternalInput").ap()         # layer 1
    hww8 = nc.dram_tensor("hww8", [2, 128, NHW, 8, 2, 2, 128], F8,
                          kind="ExternalInput").ap()           # layer 1
    hwg8 = nc.dram_tensor("hwg8", [2, 128, NHW, 4, 2, 2, 128], F8,
                          kind="ExternalInput").ap()           # L0 gate half
    bias_all = nc.dram_tensor("bias_all", [128, 2, 64], F32,
                              kind="ExternalInput").ap()
    padt8r = nc.dram_tensor("padt8r", [128, 2, BL, 2, 2, WIDTH], F8,
                            kind="ExternalInput").ap()         # layer-1 pads
    out_t = nc.dram_tensor("out_t", [L, 2, BL, 128, HC, S], F16,
                           kind="ExternalOutput").ap()

    with tile.TileContext(nc) as tc, ExitStack() as ctx:
        sb = ctx.enter_context(tc.tile_pool(name="sb", bufs=2))
        ps = ctx.enter_context(tc.tile_pool(name="ps", bufs=8, space="PSUM"))

        # ---- PE warmup: tiny self-matmuls on a memset tile keep the HAM
        # activity window busy while the startup DMAs fly, so the first
        # real matmuls run at 2.4GHz instead of the cold 1.2GHz.
        warm = sb.tile([128, 128], F16, name="warm", tag="warm", bufs=1)
        nc.vector.memset(warm[:], 0.0)
        wps = ps.tile([128, 512], F32, name="warmps", tag="ps")
        for _ in range(28):
            nc.tensor.matmul(wps[:, 0:128], warm[:], warm[:],
                             start=True, stop=True)

        # ---- persistent SBUF state ------------------------------------
        # inter-layer fp8 activations (XS-scaled), padded DoubleRow pair
        # tiles living entirely in SBUF: [ki, d, b, a, ko, pos]
        xsb = sb.tile([128, 2, BL, 2, 2, SP8], F8, name="xsb", tag="xsb",
                      bufs=1)

        xin_t = [None] * BL

        def load_xin(b, eng, part=None):
            if xin_t[b] is None:
                xin_t[b] = sb.tile([128, HC, SP], F16, name=f"xin_{b}",
                                   tag="xin", bufs=BL)
            lo, hi = (0, HC) if part is None else (2 * part, 2 * part + 2)
            eng.dma_start(xin_t[b][:, lo:hi, :], x_t[b, :, lo:hi])

        wc = [[None] * 4 for _ in range(2)]       # [d][j] -> [128, HC, 512]

        def load_wc(d, j, eng, part=None):
            if wc[d][j] is None:
                wc[d][j] = sb.tile([128, HC, 512], F16, name=f"wc_{d}{j}",
                                   tag="wc", bufs=8)
            lo, hi = (0, HC) if part is None else (2 * part, 2 * part + 2)
            eng.dma_start(wc[d][j][:, lo:hi, :], convw[d, j, :, lo:hi])

        wh = [None, None]                         # [128, NHW, 4, HC, 128]
        whg8 = [None, None]                       # [128, NHW, 4, 2, 2, 128]
        wc8 = [None, None]                        # [128, 4, 2, 2, 512]
        wh8 = [None, None]                        # [128, NHW, 8, 2, 2, 128]

        # ---- DMA issue schedule ---------------------------------------
        # Each ring (sync HWDGE / scalar=Activation HWDGE / gpsimd SWDGE)
        # moves ~100-120GB/s when all three stream (they share the ~358GB/s
        # HBM port), so the startup-critical first ~3MB (input b0/b1 + the
        # four layer-0 d0 conv tap tiles) is split into half-tiles round-
        # robined across all three rings in consumption order. The scalar
        # queue only gets early issues (it runs activations from ~12us);
        # bulk later-needed weights ride sync/gpsimd.
        btile = sb.tile([128, 2, 64], F32, name="btile", tag="btile", bufs=1)
        nc.gpsimd.dma_start(btile[:], bias_all[:])
        # layer-1 pad columns: one tiny staged DMA (early: the in-order
        # vector queue blocks on it), then vector copies into the xsb pair
        # tiles (a strided DMA straight into the pad columns explodes into
        # thousands of 3-byte descriptors).
        padstage = sb.tile([128, 2, BL, 2, 2, WIDTH], F8, name="padstage",
                           tag="padstage", bufs=1)
        nc.gpsimd.dma_start(padstage[:], padt8r[:])
        for d in range(2):
            for a in range(2):
                nc.vector.tensor_copy(xsb[:, d, :, a, :, 0:WIDTH],
                                      padstage[:, 0, :, a])
                nc.vector.tensor_copy(
                    xsb[:, d, :, a, :, WIDTH + S:WIDTH + S + WIDTH],
                    padstage[:, 1, :, a])
        load_wc(0, 0, nc.scalar, part=0)          # first matmul operands
        load_xin(0, nc.sync, part=0)
        load_wc(0, 0, nc.gpsimd, part=1)
        load_xin(0, nc.scalar, part=1)
        load_wc(0, 1, nc.sync, part=0)
        load_wc(0, 1, nc.scalar, part=1)
        load_wc(0, 2, nc.sync, part=0)
        load_wc(0, 2, nc.gpsimd, part=1)
        load_wc(0, 3, nc.scalar, part=0)
        load_wc(0, 3, nc.sync, part=1)
        load_xin(1, nc.sync)

        whg8[0] = sb.tile([128, NHW, 4, 2, 2, 128], F8, name="whg8_0",
                          tag="whg8", bufs=2)
        nc.gpsimd.dma_start(whg8[0][:], hwg8[0])
        wh[0] = sb.tile([128, NHW, 4, HC, 128], F16, name="wh_0", tag="wh",
                        bufs=2)
        nc.gpsimd.dma_start(wh[0][:], hww[0])

        load_xin(2, nc.sync)
        load_xin(3, nc.sync)
        for j in range(4):
            load_wc(1, j, nc.sync)
        wh[1] = sb.tile([128, NHW, 4, HC, 128], F16, name="wh_1", tag="wh",
                        bufs=2)
        nc.sync.dma_start(wh[1][:], hww[1])
        whg8[1] = sb.tile([128, NHW, 4, 2, 2, 128], F8, name="whg8_1",
                          tag="whg8", bufs=2)
        nc.gpsimd.dma_start(whg8[1][:], hwg8[1])
        for d in range(2):
            wc8[d] = sb.tile([128, 4, 2, 2, 512], F8, name=f"wc8_{d}",
                             tag="wc8", bufs=2)
            nc.gpsimd.dma_start(wc8[d][:], convw8[d])
            wh8[d] = sb.tile([128, NHW, 8, 2, 2, 128], F8, name=f"wh8_{d}",
                             tag="wh8", bufs=2)
            nc.gpsimd.dma_start(wh8[d][:], hww8[d])

        def bias(d, base, i):
            return btile[:, d, base + i:base + i + 1]

        # ---- layer-0 iteration chunks (fp16) --------------------------
        def l0_chunks(d, b):
            st = {}
            off = 0 if d == 0 else WIDTH

            def conv():
                xin = xin_t[b]
                xbig = sb.tile([128, HC, 512], F16, name=f"hf_{d}{b}",
                               tag="hf", bufs=3)
                h8p = [None, None]
                for oc in range(4):
                    pt = ps.tile([128, 512], F32, name=f"cps_{d}{b}{oc}",
                                 tag="ps")
                    k = 0
                    for j in range(4):
                        for hc in range(HC):
                            nc.tensor.matmul(
                                pt[:],
                                wc[d][j][:, hc, oc * 128:(oc + 1) * 128],
                                xin[:, hc, off + j:off + j + S],
                                start=(k == 0), stop=(k == 15))
                            k += 1
                    # fp16 drain FIRST: highway-0's nonlin matmuls read it
                    nc.scalar.activation(xbig[:, oc, :], pt[:], AF.Relu,
                                         bias=bias(d, B_CB, oc))
                    if h8p[oc // 2] is None:
                        h8p[oc // 2] = sb.tile(
                            [128, 2, 512], F8, name=f"h8l0_{d}{b}{oc // 2}",
                            tag="h8", bufs=8)
                    # XS-scaled fp8 copy feeds the fp8 gate matmuls
                    nc.scalar.activation(h8p[oc // 2][:, oc % 2, :], pt[:],
                                         AF.Relu, bias=bias(d, B_CB8L0, oc),
                                         scale=XS)
                st["x"] = xbig
                st["h8"] = h8p

            def hw(jh):
                xbig, h8p = st["x"], st["h8"]
                rbig = sb.tile([128, HC, 512], F16, tag="rt", bufs=2,
                               name=f"rt_{d}{b}{jh}")
                gbig = sb.tile([128, HC, 512], F16, tag="gt", bufs=2,
                               name=f"gt_{d}{b}{jh}")
                xobig = sb.tile([128, HC, 512], F16, tag=f"xo{jh}", bufs=3,
                                name=f"xo_{d}{b}{jh}")
                h8p_new = [None, None]
                for half in range(2):
                    for gc in (0, 4, 1, 5) if half == 0 else (2, 6, 3, 7):
                        pt = ps.tile([128, 512], F32,
                                     tag="ps", name=f"hps_{d}{b}{jh}{gc}")
                        bi = jh * 8 + gc
                        if gc < 4:
                            for hc in range(HC):
                                nc.tensor.matmul(
                                    pt[:], wh[d][:, jh, gc, hc, :],
                                    xbig[:, hc, :],
                                    start=(hc == 0), stop=(hc == HC - 1))
                            nc.scalar.activation(rbig[:, gc, :], pt[:],
                                                 AF.Relu,
                                                 bias=bias(d, B_HB, bi))
                        else:
                            # gate half in fp8 DoubleRow: PSUM = HWS*XS*z
                            for a in range(2):
                                nc.tensor.matmul(
                                    pt[:], whg8[d][:, jh, gc - 4, a, :, :],
                                    h8p[a][:],
                                    start=(a == 0), stop=(a == 1),
                                    perf_mode=DR)
                            nc.scalar.activation(gbig[:, gc - 4, :], pt[:],
                                                 AF.Sigmoid,
                                                 bias=bias(d, B_HB, bi),
                                                 scale=1.0 / (HWS * XS))
                    hs = slice(2 * half, 2 * half + 2)
                    nc.vector.tensor_sub(xobig[:, hs, :], xbig[:, hs, :],
                                         rbig[:, hs, :])
                    nc.vector.tensor_mul(xobig[:, hs, :], gbig[:, hs, :],
                                         xobig[:, hs, :])
                    nc.vector.tensor_add(xobig[:, hs, :], xobig[:, hs, :],
                                         rbig[:, hs, :])
                    if jh < NHW - 1:
                        h8p_new[half] = sb.tile(
                            [128, 2, 512], F8, tag="h8", bufs=8,
                            name=f"x18l0_{d}{b}{half}")
                        nc.vector.tensor_scalar_mul(
                            h8p_new[half][:], xobig[:, hs, :], XS)
                    else:
                        # XS-scaled fp8 copy straight into the padded
                        # layer-1 input pair tile (no DRAM roundtrip)
                        nc.vector.tensor_scalar_mul(
                            xsb[:, d, b, half, :, WIDTH:WIDTH + S],
                            xobig[:, hs, :], XS)
                st["x"] = xobig
                if jh < NHW - 1:
                    st["h8"] = h8p_new
                else:
                    nc.gpsimd.dma_start(out_t[0, d, b], xobig[:])

            return conv, (lambda: hw(0)), (lambda: hw(1))

        # ---- layer-1 iteration chunks (fp8 DoubleRow) -----------------
        def l1_chunks(d, b, tail=False):
            st = {}
            off = 0 if d == 0 else WIDTH

            def conv():
                xbig = sb.tile([128, HC, 512], F16, name=f"hf1_{d}{b}",
                               tag="hf", bufs=3)
                h8p = [None, None]        # fp8 XS-scaled pair tiles
                for oc in range(4):
                    pt = ps.tile([128, 512], F32, name=f"cps1_{d}{b}{oc}",
                                 tag="ps")
                    k = 0
                    for j in range(4):
                        for a in range(2):
                            nc.tensor.matmul(
                                pt[:],
                                wc8[d][:, j, a, :, oc * 128:(oc + 1) * 128],
                                xsb[:, d, b, a, :, off + j:off + j + S],
                                start=(k == 0), stop=(k == 7), perf_mode=DR)
                            k += 1
                    if h8p[oc // 2] is None:
                        h8p[oc // 2] = sb.tile(
                            [128, 2, 512], F8, name=f"h8_{d}{b}{oc // 2}",
                            tag="h8", bufs=8)
                    # fp8 copy FIRST -- it feeds the next matmuls.
                    # XS*h = relu(psum/CWS + XS*bias)  (PSUM = CWS*XS*z)
                    nc.scalar.activation(h8p[oc // 2][:, oc % 2, :], pt[:],
                                         AF.Relu, bias=bias(d, B_CB8, oc),
                                         scale=XS / (CWS * XS))
                    # D*h = relu(psum*(HWS/CWS) + D*bias)
                    nc.scalar.activation(xbig[:, oc, :], pt[:], AF.Relu,
                                         bias=bias(d, B_CB11, oc),
                                         scale=HWS / CWS)
                st["x"] = xbig
                st["h8"] = h8p

            def hw(jh):
                xbig, h8p = st["x"], st["h8"]
                rbig = sb.tile([128, HC, 512], F16, tag="rt", bufs=2,
                               name=f"rt1_{d}{b}{jh}")
                gbig = sb.tile([128, HC, 512], F16, tag="gt", bufs=2,
                               name=f"gt1_{d}{b}{jh}")
                xobig = sb.tile([128, HC, 512], F16, tag=f"xo{jh}", bufs=3,
                                name=f"xo1_{d}{b}{jh}")
                h8p_new = [None, None]
                final = jh == NHW - 1
                for half in range(2):
                    for gc in (0, 4, 1, 5) if half == 0 else (2, 6, 3, 7):
                        pt = ps.tile([128, 512], F32,
                                     tag="ps", name=f"hps1_{d}{b}{jh}{gc}")
                        for a in range(2):
                            nc.tensor.matmul(
                                pt[:], wh8[d][:, jh, gc, a, :, :],
                                h8p[a][:],
                                start=(a == 0), stop=(a == 1), perf_mode=DR)
                        bi = jh * 8 + gc
                        if gc < 4:
                            # VectorE drains the nonlin PSUMs: D*r. In the
                            # tail iteration they go to ScalarE instead so
                            # the epilogue's PSUM slots free up without
                            # queueing behind the vector combine backlog.
                            if tail:
                                nc.scalar.activation(
                                    rbig[:, gc, :], pt[:], AF.Relu,
                                    bias=bias(d, B_HB11, bi))
                            else:
                                nc.vector.tensor_scalar(
                                    rbig[:, gc, :], pt[:],
                                    bias(d, B_HB11, bi),
                                    0.0, ALU.add, ALU.max)
                        else:
                            nc.scalar.activation(gbig[:, gc - 4, :], pt[:],
                                                 AF.Sigmoid,
                                                 bias=bias(d, B_HB1, bi),
                                                 scale=1.0 / (HWS * XS))
                    hs = slice(2 * half, 2 * half + 2)
                    nc.vector.tensor_sub(xobig[:, hs, :], xbig[:, hs, :],
                                         rbig[:, hs, :])
                    nc.vector.tensor_mul(xobig[:, hs, :], gbig[:, hs, :],
                                         xobig[:, hs, :])
                    nc.vector.tensor_add(xobig[:, hs, :], xobig[:, hs, :],
                                         rbig[:, hs, :])
                    if not final:
                        h8p_new[half] = sb.tile(
                            [128, 2, 512], F8, tag="h8", bufs=8,
                            name=f"x18_{d}{b}{half}")
                        # back to XS-domain fp8 on the ScalarE
                        nc.scalar.activation(h8p_new[half][:],
                                             xobig[:, hs, :],
                                             AF.Copy, scale=1.0 / HWS)
                    elif tail:
                        # split the final output DMA across two queues;
                        # half 0 goes on sync so the waiting dma_start
                        # doesn't block gpsimd's half-1 combine ops
                        eng = nc.sync if half == 0 else nc.gpsimd
                        eng.dma_start(out_t[1, d, b, :, hs, :],
                                      xobig[:, hs, :])
                st["x"] = xobig
                if not final:
                    st["h8"] = h8p_new
                elif not tail:
                    nc.gpsimd.dma_start(out_t[1, d, b], xobig[:])

            return conv, (lambda: hw(0)), (lambda: hw(1))

        # ---- software-pipelined schedule ------------------------------
        # phase A: layer-0 d=0 (tensor-heavy; scalar/vector have slack).
        # It opens with two back-to-back convs so only conv weights are
        # needed in the first ~36us (the highway weights ride the slow
        # queue), and keeps each hw chunk's consumer a chunk away so the
        # combine->cast latency at each sublayer boundary is hidden.
        pa = [l0_chunks(0, b) for b in range(BL)]
        pa[0][0](); pa[1][0]()                  # c0 c1
        pa[0][1](); pa[1][1]()                  # j0_0 j0_1
        pa[2][0]()                              # c2
        pa[0][2](); pa[2][1]()                  # j1_0 j0_2
        pa[3][0]()                              # c3
        pa[1][2](); pa[3][1]()                  # j1_1 j0_3
        pa[2][2](); pa[3][2]()                  # j1_2 j1_3
        # prologue: L = l0d1(0), A = l1d0(0); L's chunks cover A's
        # combine latencies (and phase A's last hw1)
        Lc, Lj0, Lj1 = l0_chunks(1, 0)
        Ac, Aj0, Aj1 = l1_chunks(0, 0)
        Lc(); Ac(); Lj0(); Aj0(); Lj1(); Aj1()
        # steady state: per group, L = l0d1(b), A = l1d0(b),
        # B = l1d1(b-1); B's last highway spills into the next group
        pending = None
        pending_Aj1 = None
        for b in range(1, BL):
            last = b == BL - 1
            Lc, Lj0, Lj1 = l0_chunks(1, b)
            Ac, Aj0, Aj1 = l1_chunks(0, b)
            Bc, Bj0, Bj1 = l1_chunks(1, b - 1)
            Lc()
            if pending is not None:
                pending()
            Ac(); Lj0(); Aj0(); Bc()
            if not last:
                Aj1()
            Lj1(); Bj0()
            pending = Bj1
            if last:
                pending_Aj1 = Aj1
        # epilogue: E = l1d1(3). Deferred chunks (Aj1 of b=3, Bj1 of b=2)
        # are slotted between E's chunks so each combine->cast latency is
        # covered by another chunk's matmuls.
        Ec, Ej0, Ej1 = l1_chunks(1, BL - 1, tail=True)
        Ec()
        pending_Aj1()
        Ej0()
        if pending is not None:
            pending()
        Ej1()

    nc.compile()
    _CACHE["nc"] = nc
    return nc


def _prep_shared(fwd_pad, bwd_pad, fwd_w, fwd_b, bwd_w, bwd_b,
                 fwd_hw_w, fwd_hw_b, bwd_hw_w, bwd_hw_b):
    f32 = np.float32
    convw = np.empty((2, 4, 128, HC, 512), np.float16)
    convw8 = np.empty((2, 128, 4, 2, 2, 512), E4NP)
    hww = np.empty((2, 128, NHW, 4, HC, 128), np.float16)
    hww8 = np.empty((2, 128, NHW, 8, 2, 2, 128), E4NP)
    hwg8 = np.empty((2, 128, NHW, 4, 2, 2, 128), E4NP)
    bias_all = np.empty((128, 2, 64), f32)
    padt8r = np.empty((128, 2, BL, 2, 2, WIDTH), E4NP)
    for d, (w, bia, hw_w, hw_b) in enumerate(
            [(fwd_w, fwd_b, fwd_hw_w, fwd_hw_b),
             (bwd_w, bwd_b, bwd_hw_w, bwd_hw_b)]):
        # layer 0 fp16: w[0]: [512o, 2048=(j,hc,p)] -> [j, p, hc, o]
        convw[d] = w[0].reshape(512, 4, HC, 128).transpose(1, 3, 2, 0)
        # layer 1 fp8 DoubleRow: [512o, (j,a,ko,ki)] -> [ki, j, a, ko, o]
        convw8[d] = (w[1].reshape(512, 4, 2, 2, 128)
                     .transpose(4, 1, 2, 3, 0) * CWS).astype(E4NP)
        cb = [bia[li].reshape(4, 128).T for li in range(L)]
        bias_all[:, d, B_CB:B_CB + 4] = cb[0]
        bias_all[:, d, B_CB8L0:B_CB8L0 + 4] = cb[0] * XS
        bias_all[:, d, B_CB11:B_CB11 + 4] = cb[1] * (HWS * XS)
        bias_all[:, d, B_CB8:B_CB8 + 4] = cb[1] * XS
        for li, base in ((0, B_HB), (1, B_HB1)):
            for jh in range(NHW):
                bias_all[:, d, base + jh * 8:base + (jh + 1) * 8] = \
                    hw_b[li, jh].reshape(8, 128).T
        for jh in range(NHW):
            bias_all[:, d, B_HB11 + jh * 8:B_HB11 + (jh + 1) * 8] = \
                hw_b[1, jh].reshape(8, 128).T * (HWS * XS)
        for jh in range(NHW):
            # layer-0 nonlin half: [512=(gc,gi), 512=(hc,p)] -> [p,gc,hc,gi]
            hww[d, :, jh] = (hw_w[0, jh][:H].reshape(4, 128, HC, 128)
                             .transpose(3, 0, 2, 1))
            # layer 1 fp8: [(gc,m), (a,ko,ki)] -> [ki, gc, a, ko, m]
            hww8[d, :, jh] = (hw_w[1, jh].reshape(8, 128, 2, 2, 128)
                              .transpose(4, 0, 2, 3, 1) * HWS).astype(E4NP)
            # layer-0 gate half (rows H:2H) in the same fp8 layout
            hwg8[d, :, jh] = (hw_w[0, jh][H:].reshape(4, 128, 2, 2, 128)
                              .transpose(4, 0, 2, 3, 1) * HWS).astype(E4NP)
    # layer-1 pads: [ki, front/back, b(replicated), a, ko, 3], XS-scaled
    for i, pad in enumerate((fwd_pad, bwd_pad)):
        p = (np.asarray(pad)[1].T.reshape(2, 2, 128, WIDTH)
             .transpose(2, 0, 1, 3) * XS).astype(E4NP)      # [ki, a, ko, 3]
        padt8r[:, i] = np.broadcast_to(p[:, None], (128, BL, 2, 2, WIDTH))
    return dict(convw=convw, convw8=convw8, hww=hww, hww8=hww8, hwg8=hwg8,
                bias_all=bias_all, padt8r=padt8r)


def kernel(inputs, fwd_pad, bwd_pad, fwd_w, fwd_b, bwd_w, bwd_b,
           fwd_hw_w, fwd_hw_b, bwd_hw_w, bwd_hw_b, _trace=False):
    nc = _build()
    shared = _prep_shared(
        np.asarray(fwd_pad), np.asarray(bwd_pad),
        np.asarray(fwd_w), np.asarray(fwd_b),
        np.asarray(bwd_w), np.asarray(bwd_b),
        np.asarray(fwd_hw_w), np.asarray(fwd_hw_b),
        np.asarray(bwd_hw_w), np.asarray(bwd_hw_b))
    x = np.asarray(inputs, dtype=np.float32)

    in_maps = []
    for c in range(NCORES):
        xs = x[c * BL:(c + 1) * BL].transpose(0, 2, 1)  # [BL, H, S]
        xc = np.empty((BL, H, SP), np.float16)
        xc[:, :, WIDTH:WIDTH + S] = xs
        xc[:, :, 0:WIDTH] = np.asarray(fwd_pad)[0].T[None]
        xc[:, :, WIDTH + S:SP] = np.asarray(bwd_pad)[0].T[None]
        # [BL, H, SP] -> [BL, 128, HC, SP]
        xc = np.ascontiguousarray(
            xc.reshape(BL, HC, 128, SP).transpose(0, 2, 1, 3))
        in_maps.append({"x_t": xc, **shared})

    res = bass_utils.run_bass_kernel_spmd(
        nc, in_maps, core_ids=list(range(NCORES)), trace=_trace)

    out = np.empty((L, B, S, 2 * H), np.float32)
    for c in range(NCORES):
        o = res.results[c]["out_t"].astype(np.float32)
        o[1] /= HWS * XS   # layer 1 is computed in the HWS*XS domain
        # o: [L, 2, BL, 128, HC, S] -> [BL, S, H] per (L, dir)
        for li in range(L):
            out[li, c * BL:(c + 1) * BL, :, :H] = \
                o[li, 0].transpose(0, 3, 2, 1).reshape(BL, S, H)
            out[li, c * BL:(c + 1) * BL, :, H:] = \
                o[li, 1].transpose(0, 3, 2, 1).reshape(BL, S, H)
    if _trace:
        kernel.last_exec_time_ns = res.exec_time_ns
        kernel.last_trace = (res.instructions_and_trace[1]
                             if res.instructions_and_trace else None)
        kernel.last_res = res
    return out
